# revision 2
# baseline (speedup 1.0000x reference)
"""
Trainium2 Bass kernel v8 for nn_CVXPolicy_DoubleIntegrator
(131072 x 192 -> 131072 x 96), 8 cores data-parallel.

Math:  p = MLP(concat([t,z])) (tanh x2, hidden 100); q = -(p@S + b3@S);
       u* = q/(1+s), s(1+s)^2 = ||q||^2 (Cardano, bit-trick cbrt).

v8 = continuous global block stream (the big one):
  - ph1 emits one block per global tick with NO rep boundary: stage X of
    rep r block j sits at global tick 32r + j + OFF_X. The per-rep tail
    (r-lag + solve + ph3 windows) that idled PE/ACT ~10-14 ticks per rep
    in v5-v7 (and triggered PE P-state down-clocks) is gone.
  - qsb (persistent q) and rps are DOUBLE-buffered by rep parity, which
    decouples ph3 (prev rep's u=q*c) from qm overwrites: ut(r) reads
    qsb[r%2] long before qm(r+2) rewrites it.
  - solve: [128,128] layout (4-way one-hot r-reduce), chain almost
    entirely on Pool with magic-number reciprocals (no DVE blocking,
    no ACT tables - ACT runs tanh only, one table load ever).
  - c broadcast: DRAM scratch roundtrip + per-block broadcast DMA.
  Engine budget/rep: ACT 39.2 | PE ~36 | DVE ~35 | Pool ~36 | DMA ~35.
"""

import os
import sys
import numpy as np
from contextlib import ExitStack

for _p in ("/opt/trn_rl_repo", "/root/.axon_site/_ro/trn_rl_repo"):
    if _p not in sys.path and os.path.isdir(_p):
        sys.path.append(_p)

B_TOTAL = 131072
N_CORES = 8
BC = B_TOTAL // N_CORES          # 16384 rows per core
SD = 192
CD = 96
HID = 100
BLKN = 512
NBLK = BC // BLKN                # 32
SUPER = 4                        # blocks per DMA superblock
NSUP = NBLK // SUPER             # 8
SUPN = SUPER * BLKN              # 2048
NPAIR = NBLK // 2                # 16
REPEAT = int(os.environ.get("K_REPEAT", "1"))
C16 = 400                        # w1a|w1bt|w2|w3|ones96

_PROG_CACHE = {}


def _scatter_matrix():
    n = 32
    u_idx = np.arange(n)
    p_idx = np.concatenate([3 * np.arange(1, n + 1), 4 * np.arange(1, n + 1),
                            5 * np.arange(1, n + 1)])
    uu_idx = np.concatenate([u_idx, 2 * u_idx, 3 * u_idx])
    S = np.zeros((SD, CD), np.float64)
    for pi, ui in zip(p_idx, uu_idx):
        S[pi, ui] += 1.0
    return S


def _build_program():
    import concourse.mybir as mybir
    from concourse import bacc
    from concourse.tile import TileContext

    f32 = mybir.dt.float32
    f16 = mybir.dt.float16
    i32 = mybir.dt.int32
    FT = mybir.ActivationFunctionType
    ALU = mybir.AluOpType
    fnp = np.float32

    nc = bacc.Bacc("TRN2", target_bir_lowering=False, debug=False,
                   num_devices=N_CORES)

    def din(name, shape, dt):
        return nc.dram_tensor(name, shape, dt, kind="ExternalInput").ap()

    xa_d = din("xa", [NSUP, 128, SUPN], f16)
    xbt_d = din("xbt", [NSUP, 65, SUPN], f16)
    cpk16_d = din("cpk16", [128, C16], f16)
    cpk32_d = din("cpk32", [128, 8], f32)
    rsel_d = din("rsel", [96, 32 * 32], f16)
    rselB_d = din("rselB", [96, 64 * 32], f16)
    uT_d = nc.dram_tensor("uT", [NSUP, CD, SUPN], f16,
                          kind="ExternalOutput").ap()
    scr_d = nc.dram_tensor("scr", [2, 32, 4, 128], f16, kind="Internal").ap()

    UTD = int(os.environ.get("K_UTD", "5"))     # ut pairs on DVE (of 16)
    R_LAG = int(os.environ.get("K_RLAG", "8"))
    EMIT_P2 = NBLK - 1 + R_LAG                  # after a rep's last r-mm
    RATE = int(os.environ.get("K_RATE", "3"))   # solve ops emitted per tick
    NJOBS = 38                                  # solve chain length
    # cb must be emitted AFTER the scr write drains (same queue => ordered)
    OFFCB_MIN = NBLK + R_LAG + (NJOBS + RATE - 1) // RATE + 2
    OFFCB = max(int(os.environ.get("K_OFFCB", "62")), OFFCB_MIN)
    OFFUT = OFFCB + 2
    TAIL = OFFUT + NBLK + 16

    with TileContext(nc) as tc, ExitStack() as ctx:
        ctx.enter_context(nc.allow_low_precision(
            reason="fp16 wire dtypes; accumulation stays fp32 in PSUM"))
        consts = ctx.enter_context(tc.tile_pool(name="consts", bufs=1))
        cpk16 = consts.tile([128, C16], f16)
        cpk32 = consts.tile([128, 8], f32)
        rsel = consts.tile([96, 32 * 32], f16)
        rselB = consts.tile([96, 64 * 32], f16)
        nc.scalar.dma_start(out=cpk16[:], in_=cpk16_d[:])
        nc.scalar.dma_start(out=cpk32[:], in_=cpk32_d[:])
        nc.scalar.dma_start(out=rsel[:], in_=rsel_d[:])
        nc.scalar.dma_start(out=rselB[:], in_=rselB_d[:])
        w1a = cpk16[0:128, 0:100]
        w1bt = cpk16[0:65, 100:200]
        w2 = cpk16[0:100, 200:300]
        w3 = cpk16[0:100, 300:396]
        b1 = cpk32[0:100, 0:1]
        b2 = cpk32[0:100, 1:2]
        b3 = cpk32[0:96, 3:4]

        qsb = [consts.tile([CD, BC], f16, name=f"qsb{i}") for i in range(2)]

        xpool = ctx.enter_context(tc.tile_pool(name="x", bufs=3))
        hpool = ctx.enter_context(
            tc.tile_pool(name="h", bufs=int(os.environ.get("K_HB", "6"))))
        qqpool = ctx.enter_context(
            tc.tile_pool(name="qq", bufs=int(os.environ.get("K_QB", "4"))))
        opool = ctx.enter_context(tc.tile_pool(name="o", bufs=3))
        cbp = ctx.enter_context(
            tc.tile_pool(name="cb", bufs=int(os.environ.get("K_CB", "3"))))
        ppool = ctx.enter_context(tc.tile_pool(name="p2", bufs=2))
        mm1 = ctx.enter_context(tc.tile_pool(name="mm1", bufs=2, space="PSUM"))
        mm2 = ctx.enter_context(tc.tile_pool(name="mm2", bufs=2, space="PSUM"))
        mmq = ctx.enter_context(tc.tile_pool(name="mmq", bufs=2, space="PSUM"))
        rpsp = ctx.enter_context(tc.tile_pool(name="rps", bufs=2,
                                              space="PSUM"))

        K2 = float(fnp(253.0 * 8388608.0 / 3.0))

        def phase2(rps, cstack, rep):
            """Cardano solve on [128,128]: op 1 (PSUM read) on DVE, the
            rest on Pool with magic-number reciprocals. Final job DMAs
            cstack to DRAM scratch (parity) on the scalar queue (same
            queue as the cb broadcast reads -> ordered)."""
            P = slice(0, 128)
            A = slice(0, 128)

            def tl(tag):
                return ppool.tile([128, 128], f32, tag=tag, name=tag)

            t_qp, t_sq, t_rc, t_g, t_u3 = (tl("p2a"), tl("p2b"), tl("p2c"),
                                           tl("p2d"), tl("p2e"))
            t_fi, t_u0, t_s3, t_nm, t_dn = (tl("p2f"), tl("p2g2"), tl("p2h"),
                                            tl("p2i"), tl("p2j"))
            t_u1, t_iu, t_y, t_t = tl("p2k"), tl("p2l"), tl("p2m"), tl("p2n")
            KR = float(fnp(np.float32(0x7EF311C3)))
            jb = []

            def precip(dst, srcf, polish, out_ap=None, half=False):
                # dst ~= 1/src (or 0.5/src when half): bitcast magic +
                # Newton polish x' = x*(2 - d*x), d = src (or 2*src)
                kr = KR - (8388608.0 if half else 0.0)
                s1 = -2.0 if half else -1.0
                jb.append(lambda: nc.gpsimd.tensor_scalar(
                    out=t_fi[P, A], in0=srcf[P, A].bitcast(i32),
                    scalar1=-1.0, scalar2=kr, op0=ALU.mult, op1=ALU.add))
                jb.append(lambda: nc.gpsimd.tensor_scalar(
                    out=dst[P, A].bitcast(i32), in0=t_fi[P, A],
                    scalar1=0.0, scalar2=None, op0=ALU.add))
                for it in range(polish):
                    last = it == polish - 1
                    jb.append(lambda: nc.gpsimd.tensor_tensor(
                        out=t_t[P, A], in0=srcf[P, A], in1=dst[P, A],
                        op=ALU.mult))
                    jb.append(lambda: nc.gpsimd.tensor_scalar(
                        out=t_t[P, A], in0=t_t[P, A], scalar1=s1,
                        scalar2=2.0, op0=ALU.mult, op1=ALU.add))
                    o = out_ap if (last and out_ap is not None) else dst
                    jb.append(lambda o=o: nc.gpsimd.tensor_tensor(
                        out=o[P, A], in0=dst[P, A], in1=t_t[P, A],
                        op=ALU.mult))

            jb.append(lambda: nc.vector.tensor_scalar(
                out=t_qp[P, A], in0=rps[P, A],
                scalar1=float(fnp(2.0 / 27.0)), scalar2=None, op0=ALU.add))
            jb.append(lambda: nc.gpsimd.tensor_tensor(
                out=t_sq[P, A], in0=t_qp[P, A], in1=t_qp[P, A], op=ALU.mult))
            precip(t_rc, t_sq, 0)
            jb.append(lambda: nc.gpsimd.tensor_scalar(
                out=t_g[P, A], in0=t_rc[P, A],
                scalar1=float(fnp(-2.0 / 729.0)), scalar2=2.0,
                op0=ALU.mult, op1=ALU.add))
            jb.append(lambda: nc.gpsimd.tensor_tensor(
                out=t_u3[P, A], in0=t_qp[P, A], in1=t_g[P, A], op=ALU.mult))
            jb.append(lambda: nc.gpsimd.tensor_scalar(
                out=t_fi[P, A], in0=t_u3[P, A].bitcast(i32),
                scalar1=float(fnp(1.0 / 3.0)), scalar2=K2,
                op0=ALU.mult, op1=ALU.add))
            jb.append(lambda: nc.gpsimd.tensor_scalar(
                out=t_u0[P, A].bitcast(i32), in0=t_fi[P, A],
                scalar1=0.0, scalar2=None, op0=ALU.add))
            jb.append(lambda: nc.gpsimd.tensor_tensor(
                out=t_s3[P, A], in0=t_u0[P, A], in1=t_u0[P, A], op=ALU.mult))
            jb.append(lambda: nc.gpsimd.tensor_tensor(
                out=t_s3[P, A], in0=t_s3[P, A], in1=t_u0[P, A], op=ALU.mult))
            jb.append(lambda: nc.gpsimd.tensor_tensor(
                out=t_nm[P, A], in0=t_s3[P, A], in1=t_u3[P, A], op=ALU.add))
            jb.append(lambda: nc.gpsimd.tensor_scalar(
                out=t_dn[P, A], in0=t_s3[P, A], scalar1=4.0, scalar2=None,
                op0=ALU.mult))
            jb.append(lambda: nc.gpsimd.tensor_tensor(
                out=t_dn[P, A], in0=t_dn[P, A], in1=t_u3[P, A], op=ALU.add))
            precip(t_s3, t_dn, 2)            # t_s3 now holds 1/dn
            jb.append(lambda: nc.gpsimd.tensor_tensor(
                out=t_nm[P, A], in0=t_nm[P, A], in1=t_s3[P, A], op=ALU.mult))
            jb.append(lambda: nc.gpsimd.tensor_tensor(
                out=t_u1[P, A], in0=t_u0[P, A], in1=t_nm[P, A], op=ALU.mult))
            precip(t_iu, t_u1, 1)
            jb.append(lambda: nc.gpsimd.tensor_scalar(
                out=t_iu[P, A], in0=t_iu[P, A],
                scalar1=float(fnp(1.0 / 36.0)), scalar2=float(fnp(1.0 / 6.0)),
                op0=ALU.mult, op1=ALU.add))
            jb.append(lambda: nc.gpsimd.tensor_tensor(
                out=t_y[P, A], in0=t_u1[P, A], in1=t_iu[P, A], op=ALU.add))
            precip(t_dn, t_y, 2, out_ap=cstack, half=True)
            jb.append(lambda: nc.scalar.dma_start(
                out=scr_d[rep % 2].rearrange("j k n -> k j n"),
                in_=cstack[:]))
            return jb

        # ----------- continuous global block-stream emission -------------
        # stage X of rep r, block j happens at global tick 32*r + j + OFF_X
        st_x, st_h1p, st_h1, st_h2p, st_h2 = {}, {}, {}, {}, {}
        st_qp, st_qq, st_cb, st_ut, st_rps = {}, {}, {}, {}, {}
        pending = []

        def _prefetch(g):
            # g = global superblock index
            if g >= REPEAT * NSUP or g in st_x:
                return
            xa = xpool.tile([128, SUPN], f16, tag="xa", name="xa")
            nc.sync.dma_start(out=xa[:], in_=xa_d[g % NSUP])
            xb = xpool.tile([65, SUPN], f16, tag="xb", name="xb")
            nc.sync.dma_start(out=xb[:], in_=xbt_d[g % NSUP])
            st_x[g] = (xa, xb)

        def l1_emit(bg):
            g, so = divmod(bg, SUPER)
            if so == 0:
                if g == 0:
                    _prefetch(0)
                    _prefetch(1)
                _prefetch(g + 2)
                if g >= 1:
                    st_x.pop(g - 1, None)
            xa, xb = st_x[g]
            ns = slice(so * BLKN, (so + 1) * BLKN)
            h1p = mm1.tile([HID, BLKN], f32, tag="h1p", name="h1p")
            nc.tensor.matmul(h1p[:], w1a, xa[:, ns], start=True, stop=False)
            nc.tensor.matmul(h1p[:], w1bt, xb[0:65, ns],
                             start=False, stop=True)
            st_h1p[bg] = h1p

        def tanh1_emit(bg):
            h1 = hpool.tile([HID, BLKN], f16, tag="h1", name="h1")
            nc.scalar.activation(out=h1[:], in_=st_h1p.pop(bg)[:],
                                 func=FT.Tanh, bias=b1)
            st_h1[bg] = h1

        def l2_emit(bg):
            h2p = mm2.tile([HID, BLKN], f32, tag="h2p", name="h2p")
            nc.tensor.matmul(h2p[:], w2, st_h1.pop(bg)[:], start=True,
                             stop=True)
            st_h2p[bg] = h2p

        def tanh2_emit(bg):
            h2 = hpool.tile([HID, BLKN], f16, tag="h2", name="h2")
            nc.scalar.activation(out=h2[:], in_=st_h2p.pop(bg)[:],
                                 func=FT.Tanh, bias=b2)
            st_h2[bg] = h2

        def l3_emit(bg):
            qp = mmq.tile([CD, BLKN], f32, tag="qp", name="qp")
            nc.tensor.matmul(qp[:], w3, st_h2.pop(bg)[:], start=True,
                             stop=True)
            st_qp[bg] = qp

        def qm_emit(bg):
            rep, j = divmod(bg, NBLK)
            n0 = j * BLKN
            nc.vector.tensor_scalar(out=qsb[rep % 2][:, n0:n0 + BLKN],
                                    in0=st_qp.pop(bg)[:], scalar1=b3,
                                    scalar2=None, op0=ALU.add)

        def qs_emit(bg):
            # pair op (bg odd): q^2 fp16 2x
            rep, j = divmod(bg, NBLK)
            n0 = (j - 1) * BLKN
            qq = qqpool.tile([CD, 2 * BLKN], f16, tag="qq", name="qq")
            nc.vector.tensor_tensor(out=qq[:],
                                    in0=qsb[rep % 2][:, n0:n0 + 2 * BLKN],
                                    in1=qsb[rep % 2][:, n0:n0 + 2 * BLKN],
                                    op=ALU.mult)
            st_qq[bg // 2] = qq

        def r_emit(bg):
            rep, j = divmod(bg, NBLK)
            if j == 0:
                st_rps[rep] = rpsp.tile([128, 128], f32, tag="rps",
                                        name="rps")
            rps = st_rps[rep]
            qq = st_qq[bg // 2]
            h = j % 2
            c0 = 512 * h
            st = (j == 0)
            sp = (j == NBLK - 1)
            nc.tensor.matmul(rps[64:128, :], rselB[:, 64 * j:64 * j + 64],
                             qq[:, c0 + 384:c0 + 512], start=st, stop=sp)
            for k in range(3):
                nc.tensor.matmul(rps[32 * k:32 * k + 32, :],
                                 rsel[:, 32 * j:32 * j + 32],
                                 qq[:, c0 + 128 * k:c0 + 128 * k + 128],
                                 start=st, stop=sp)
            if h == 1:
                st_qq.pop(bg // 2)

        def cb_emit(bg):
            # one broadcast DMA per TWO superblocks (8 blocks) - 4
            # issues/rep on the ACT queue
            rep, j = divmod(bg, NBLK)
            if j % 8 != 0:
                return
            du = j // 8
            cb = cbp.tile([CD, 2 * SUPN], f16, tag="cb", name="cb")
            st_cb[(rep * NBLK + j) // 8] = cb
            scrv = scr_d[rep % 2].rearrange("(s q) k n -> s (q k n)", s=4)
            nc.scalar.dma_start(
                out=cb[:], in_=scrv[du:du + 1].to_broadcast([CD, 2 * SUPN]))

        def ut_emit(bg):
            # pair op (bg odd): u = q * c
            rep, j = divmod(bg, NBLK)
            p = j // 2
            su, half = divmod(p, 2)
            gsu = rep * NSUP + su
            if half == 0:
                st_ut[gsu] = opool.tile([CD, SUPN], f16, tag="ut", name="ut")
            n0 = (j - 1) * BLKN
            ns = slice(half * 2 * BLKN, (half + 1) * 2 * BLKN)
            cb = st_cb[gsu // 2]
            cns = slice((su % 2) * SUPN + half * 2 * BLKN,
                        (su % 2) * SUPN + (half + 1) * 2 * BLKN)
            eng = nc.vector if (p * UTD) % NPAIR < UTD else nc.gpsimd
            eng.tensor_tensor(out=st_ut[gsu][:, ns],
                              in0=qsb[rep % 2][:, n0:n0 + 2 * BLKN],
                              in1=cb[:, cns], op=ALU.mult)
            if half == 1:
                if su % 2 == 1:
                    st_cb.pop(gsu // 2)
                nc.sync.dma_start(out=uT_d[su], in_=st_ut.pop(gsu)[:])

        def stage(G, off, nblk_total):
            bg = G - off
            if 0 <= bg < nblk_total:
                return bg
            return None

        NB_T = NBLK * REPEAT
        GMAX = NB_T + TAIL
        for G in range(GMAX):
            bg = stage(G, 0, NB_T)
            if bg is not None:
                l1_emit(bg)
            bg = stage(G, 1, NB_T)
            if bg is not None:
                tanh1_emit(bg)
            bg = stage(G, 2, NB_T)
            if bg is not None:
                l2_emit(bg)
            bg = stage(G, 3, NB_T)
            if bg is not None:
                tanh2_emit(bg)
            bg = stage(G, 4, NB_T)
            if bg is not None:
                l3_emit(bg)
            bg = stage(G, 5, NB_T)
            if bg is not None:
                qm_emit(bg)
            bg = stage(G, 6, NB_T)
            if bg is not None and bg % 2 == 1:
                qs_emit(bg)
            bg = stage(G, R_LAG, NB_T)
            if bg is not None:
                r_emit(bg)
                rep, j = divmod(bg, NBLK)
                if j == NBLK - 1:
                    pending += phase2(st_rps.pop(rep),
                                      ppool.tile([128, 128], f16, tag="cst",
                                                 name="cstack"), rep)
            for _ in range(min(RATE, len(pending))):
                pending.pop(0)()
            bg = stage(G, OFFCB, NB_T)
            if bg is not None:
                cb_emit(bg)
            bg = stage(G, OFFUT, NB_T)
            if bg is not None and bg % 2 == 1:
                ut_emit(bg)

    nc.compile()
    return nc


def _host_constants(W1, b1, W2, b2, W3, b3):
    S = _scatter_matrix()
    f16 = np.float16
    W1 = np.asarray(W1, np.float32)
    W1z, W1t = W1[1:, :], W1[0, :]
    w3 = (-(np.asarray(W3, np.float64) @ S)).astype(np.float32)
    b3n = (-(np.asarray(b3, np.float64) @ S)).astype(np.float32)

    cpk16 = np.zeros((128, C16), f16)
    cpk16[0:128, 0:100] = W1z[0:128].astype(f16)
    cpk16[0:64, 100:200] = W1z[128:192].astype(f16)
    cpk16[64, 100:200] = W1t.astype(f16)
    cpk16[0:100, 200:300] = np.asarray(W2, np.float32).astype(f16)
    cpk16[0:100, 300:396] = w3.astype(f16)
    cpk16[0:96, 396] = 1.0

    cpk32 = np.zeros((128, 8), np.float32)
    cpk32[0:100, 0] = np.asarray(b1, np.float32)
    cpk32[0:100, 1] = np.asarray(b2, np.float32)
    cpk32[0:96, 3] = b3n

    rsel = np.zeros((96, 32 * 32), f16)
    for j in range(32):
        rsel[:, 32 * j + j] = 1.0
    rselB = np.zeros((96, 64 * 32), f16)
    for j in range(32):
        rselB[:, 64 * j + 32 + j] = 1.0
    return {"cpk16": cpk16, "cpk32": cpk32, "rsel": rsel, "rselB": rselB}


def _shard_inputs(z, t, consts):
    f16 = np.float16
    in_maps = []
    for c in range(N_CORES):
        sl = slice(c * BC, (c + 1) * BC)
        zc = np.asarray(z[sl], np.float32).astype(f16)
        tc = np.asarray(t[sl], np.float32).astype(f16).reshape(BC)
        m = dict(consts)
        xa = zc.T[0:128].reshape(128, NSUP, SUPN).transpose(1, 0, 2)
        m["xa"] = np.ascontiguousarray(xa)
        xbt = np.zeros((NSUP, 65, SUPN), f16)
        xbt[:, 0:64, :] = (zc.T[128:192]
                           .reshape(64, NSUP, SUPN).transpose(1, 0, 2))
        xbt[:, 64, :] = tc.reshape(NSUP, SUPN)
        m["xbt"] = xbt
        in_maps.append(m)
    return in_maps


def _unshard_output(uT):
    full = np.asarray(uT, np.float32).transpose(1, 0, 2).reshape(CD, BC)
    return np.ascontiguousarray(full.T)


def _get_program():
    key = (REPEAT,)
    if key not in _PROG_CACHE:
        _PROG_CACHE[key] = _build_program()
    return _PROG_CACHE[key]


def kernel(z, t, W1, b1, W2, b2, W3, b3, _trace=False):
    from concourse.bass_utils import run_bass_kernel_spmd

    consts = _host_constants(W1, b1, W2, b2, W3, b3)
    nc = _get_program()
    in_maps = _shard_inputs(np.asarray(z), np.asarray(t), consts)
    res = run_bass_kernel_spmd(nc, in_maps, list(range(N_CORES)),
                               trace=_trace)
    outs = [_unshard_output(res.results[c]["uT"]) for c in range(N_CORES)]
    u = np.concatenate(outs, axis=0).astype(np.float32)
    if _trace:
        return u, res
    return u


def _make_runner(in_maps):
    import jax
    import numpy as _np
    from jax.sharding import Mesh, PartitionSpec
    from jax.experimental.shard_map import shard_map
    import concourse.mybir as mybir
    from concourse import bass2jax

    nc = _get_program()
    bass2jax.install_neuronx_cc_hook()

    partition_name = (nc.partition_id_tensor.name
                      if nc.partition_id_tensor else None)
    in_names, out_names, out_avals, zero_outs = [], [], [], []
    for alloc in nc.m.functions[0].allocations:
        if not isinstance(alloc, mybir.MemoryLocationSet):
            continue
        name = alloc.memorylocations[0].name
        if alloc.kind == "ExternalInput":
            if name != partition_name:
                in_names.append(name)
        elif alloc.kind == "ExternalOutput":
            shape = list(alloc.tensor_shape)
            dt = mybir.dt.np(alloc.dtype)
            out_names.append(name)
            out_avals.append(jax.core.ShapedArray(shape, dt))
            zero_outs.append(_np.zeros(shape, dt))
    in_names_full = in_names + out_names
    if partition_name is not None:
        in_names_full.append(partition_name)

    def _body(*args):
        operands = list(args)
        if partition_name is not None:
            operands.append(bass2jax.partition_id_tensor())
        outs = bass2jax._bass_exec_p.bind(
            *operands,
            out_avals=tuple(out_avals),
            in_names=tuple(in_names_full),
            out_names=tuple(out_names),
            lowering_input_output_aliases=(),
            sim_require_finite=True,
            sim_require_nnan=True,
            nc=nc,
        )
        return tuple(outs)

    devices = jax.devices()[:N_CORES]
    mesh = Mesh(np.asarray(devices), ("core",))
    nin = len(in_names) + len(zero_outs)
    fn = jax.jit(shard_map(_body, mesh=mesh,
                           in_specs=(PartitionSpec("core"),) * nin,
                           out_specs=(PartitionSpec("core"),) * len(out_names),
                           check_rep=False), keep_unused=True)
    concat = [_np.concatenate([in_maps[c][n] for c in range(N_CORES)], axis=0)
              for n in in_names]
    concat += [_np.zeros((N_CORES * zz.shape[0], *zz.shape[1:]), zz.dtype)
               for zz in zero_outs]
    sh = jax.sharding.NamedSharding(mesh, PartitionSpec("core"))
    dev_in = [jax.device_put(a, sh) for a in concat]
    return fn, dev_in, out_names
